# revision 42
# baseline (speedup 1.0000x reference)
"""MultiHeadAttention (CLUSTERING softmax over query axis) on 8 Trainium2 cores.

Sharding: batch B=8, one batch element per NeuronCore (pure data parallel,
no collectives).

Per-core computation (L=1024, D=1024, H=16, HD=64):
  QT = (x_q @ Wq)^T            [d, l]   (bq dropped: cancels in softmax over q)
  KT = (x_k @ Wk + bk)^T       [d, l]
  V  = x_v @ Wv + bv           [l, d]
  per head h: ST_h[k, q] = QT_h . KT_h  (contraction over hd=64)
  E = exp(ST / 32)  with fused row-sums over q (free axis)
  r = 1/sums; V'_h[k, :] = V_h[k, :] * r_h[k]   (normalizer folded into V)
  OT_h[d, q] = sum_k V'_h[k, d] * E_h[k, q]
  y = OT^T @ Wo + bo           [l, d]

v7 structure (evolved from the HW-calibrated v6):
  - x_q/x_k/x_v transposed on the HOST (no xbar DMA transposes); x_q/x_k
    and Wq/Wk shipped as fp8e4 -- the Q/K projections run in fp8
    DoubleRow perf mode (2 k-tiles per matmul). Numerically safe: Q/K
    errors are crushed by the 1/sqrt(D) score scale + softmax.
  - V/O paths stay bf16 (fp8 there would cost ~3% rel err vs 2e-2 gate).
  - DMA order: q/k path first so the PE starts ~8us in; V projection
    deferred into the main loop (dc half 0 at hp==1, half 1 at hp==2),
    with vp moved to just before its consuming av.
  - bv/bo biases via host-broadcast [128, D] tiles + DVE adds in the
    PSUM evacuation (replaces 32 rank-1 matmuls).
"""

import math
from contextlib import ExitStack, nullcontext

import numpy as np

import concourse.bass as bass
import concourse.tile as tile
from concourse import mybir
from concourse.bass import ts

F32 = mybir.dt.float32
BF16 = mybir.dt.bfloat16
FP8 = mybir.dt.float8e4
EXP = mybir.ActivationFunctionType.Exp
COPY = mybir.ActivationFunctionType.Copy
ADD = mybir.AluOpType.add
DR = mybir.MatmulPerfMode.DoubleRow

L = 1024
D = 1024
P = 128
NT = 8  # 1024 / 128
N_CORES = 8
SCALE = 1.0 / math.sqrt(D)
N_QUEUES = 4
QTKT_BUFS = 6
WQK_BUFS = 4
USE_FP8_QK = True


# ---------------------------------------------------------------------------
# Workaround: this walrus build supports very few sync-wait commands per
# instruction. Tile's kernel-tail drain / barriers can carry more. Move
# excess waits onto same-engine NOPs inserted immediately before (engines
# execute their stream in order, so this preserves semantics).
def split_excess_waits(nc):
    f = nc.m.functions[0]
    ctr = 0
    for b in f.blocks:
        insts = b.instructions
        i = 0
        while i < len(insts):
            inst = insts[i]
            si = inst.sync_info
            limit = 0 if "Drain" in type(inst).__name__ else 1
            if si is not None and si.on_wait and len(si.on_wait) > limit:
                waits = list(si.on_wait)
                keep = waits[-limit:] if limit else []
                extra = waits[: len(waits) - limit]
                pos = i
                for j in range(0, len(extra), 1):
                    nop = mybir.InstNoOp(name=f"waitsplit-{ctr}", ins=[], outs=[])
                    ctr += 1
                    nop.engine = inst.engine
                    nop.bass_nofuse = True
                    nop.sync_info = mybir.SyncInfo(
                        on_wait=[extra[j]], on_update=[]
                    )
                    insts.insert(pos, nop)
                    pos += 1
                    i += 1
                inst.sync_info = mybir.SyncInfo(
                    on_wait=keep, on_update=list(si.on_update)
                )
            i += 1


# ---------------------------------------------------------------------------
def _emit_body(nc, tc, ctx, t):
    persist = ctx.enter_context(tc.tile_pool(name="persist", bufs=1))
    pairp = ctx.enter_context(tc.tile_pool(name="pairp", bufs=4, space="PSUM"))

    XDT = FP8 if USE_FP8_QK else BF16

    # ---- constants -------------------------------------------------------
    bk_sb = persist.tile([P, NT], F32, name="bk")
    bvb = persist.tile([P, D], BF16, name="bvb")
    bob = persist.tile([P, D], BF16, name="bob")

    # ---- persistent SBUF tensors ----------------------------------------
    xqT = persist.tile([P, NT, L], XDT, name="xqT")
    xkT = persist.tile([P, NT, L], XDT, name="xkT")
    xvT = persist.tile([P, NT, L], BF16, name="xvT")
    wv_bf = [persist.tile([P, D], BF16, name=f"wv{i}") for i in range(NT)]
    wo_bf = [persist.tile([P, D], BF16, name=f"wo{i}") for i in range(NT)]
    v_sb = [persist.tile([P, D], BF16, name=f"v{i}") for i in range(NT)]
    ot_sb = [persist.tile([P, D], BF16, name=f"ot{i}") for i in range(NT)]

    # ---- DMA issue order == PE consumption order -------------------------
    # q/k path first so the first projections/scores start ASAP, then the
    # V-projection inputs (needed from hp==1), then wo (needed at the tail).
    wqk = ctx.enter_context(tc.tile_pool(name="wqk", bufs=WQK_BUFS))
    wpairs = {}

    def fetch_w(hp):
        pair = []
        for tag, wd in (("q", t["wqr"]), ("k", t["wkr"])):
            w_t = wqk.tile([P, NT, P], XDT, name=f"w{tag}")
            nc.sync.dma_start(w_t[:], wd[ts(hp, P), :])
            pair.append(w_t)
        wpairs[hp] = pair

    fetch_w(0)
    nc.sync.dma_start(bk_sb[:], t["bk"].rearrange("(a p) -> p a", p=P))
    for ct in range(NT):
        nc.sync.dma_start(xqT[:, ct, :], t["xq"][:, ts(ct, L)])
        nc.sync.dma_start(xkT[:, ct, :], t["xk"][:, ts(ct, L)])
    fetch_w(1)
    # V inputs next: the vb0 fill chains (scheduled into late hp0 slots)
    # consume (wv[ct], xvT[ct]) pairs in ct order as they land
    for ct in range(NT):
        nc.sync.dma_start(wv_bf[ct][:], t["wv"][ts(ct, P), :])
        nc.sync.dma_start(xvT[:, ct, :], t["xv"][:, ts(ct, L)])
    nc.sync.dma_start(bvb[:], t["bvb"][:, :])
    for i in range(NT):
        nc.sync.dma_start(wo_bf[i][:], t["wo"][ts(i, P), :])
    nc.sync.dma_start(bob[:], t["bob"][:, :])

    # ---- warmup during the DMA lead-in: ramp the PE p-state with dummy
    # matmuls and pre-load the ACT Exp table so the first real score work
    # runs at full clock with no table-load stall.
    warm = persist.tile([P, 512], BF16, name="warm")
    nc.gpsimd.memset(warm[:], 1.0)
    wdum = persist.tile([1, NT], F32, name="wdum")
    with tc.tile_pool(name="warmps", bufs=1, space="PSUM") as warmps:
        wps = warmps.tile([P, 512], F32, name="wp")
        for i in range(8):
            nc.tensor.matmul(
                wps[:], warm[:, 0:P], warm[:], start=True, stop=True
            )
        nc.scalar.activation(
            wdum[0:1, :], warm[0:1, 0:NT], EXP, scale=SCALE,
        )

    # ---- pools for the main loop ----------------------------------------
    qtkt = ctx.enter_context(tc.tile_pool(name="qtkt", bufs=QTKT_BUFS))
    epool = ctx.enter_context(tc.tile_pool(name="epool", bufs=4))
    sums = ctx.enter_context(tc.tile_pool(name="sums", bufs=4))
    vppool = ctx.enter_context(tc.tile_pool(name="vppool", bufs=2))
    stq = ctx.enter_context(tc.tile_pool(name="stq", bufs=2, space="PSUM"))

    # QT/KT projection for one head pair: contraction over din=1024 as
    # fp8 DoubleRow (4 matmuls of 2 k-tiles each per 512-wide chain).
    # Returns (qt_tile, kt_tile, chunks): 8 closures of 2 matmuls each,
    # interleaved into the previous head-pair's fused loop so the
    # PSUM->SBUF combines land on DVE well before the tiles are consumed.
    def proj_chunks(hp):
        qt_t = qtkt.tile([P, L], BF16, name="qt")
        kt_t = qtkt.tile([P, L], BF16, name="kt")
        chunks = []
        wp = wpairs.pop(hp)
        for out_t, w_t, tag in ((qt_t, wp[0], "qt"), (kt_t, wp[1], "kt")):
            for lc in range(2):
                cell = {}

                def mk(cell, w_t, out_t, tag, lc, i0):
                    def emit():
                        if i0 == 0:
                            cell["ps"] = pairp.tile([P, 512], F32, name="pA")
                        ps = cell["ps"]
                        for i in (i0, i0 + 1):
                            nc.tensor.matmul(
                                ps[:],
                                w_t[:, 2 * i : 2 * i + 2, :],
                                (xqT if tag == "qt" else xkT)[
                                    :, 2 * i : 2 * i + 2, ts(lc, 512)
                                ],
                                start=(i == 0),
                                stop=(i == 3),
                                perf_mode=DR,
                            )
                        if i0 == 2:
                            if tag == "kt":
                                nc.vector.tensor_scalar_add(
                                    out_t[:, ts(lc, 512)], ps[:],
                                    bk_sb[:, hp : hp + 1],
                                )
                            else:
                                nc.vector.tensor_copy(
                                    out_t[:, ts(lc, 512)], ps[:]
                                )
                    return emit

                chunks.append(mk(cell, w_t, out_t, tag, lc, 0))
                chunks.append(mk(cell, w_t, out_t, tag, lc, 2))
        return qt_t, kt_t, chunks

    # V[lt, b-quarter] = x_v @ Wv + bv: 32 chains of 8 matmuls x 256 cols,
    # used as deadline-scheduled PE fill inside the fused loops (block b
    # covers V columns for head pairs 2b..2b+1).
    def mk_vchain(lt, b):
        def emit():
            ps = pairp.tile([P, 512], F32, name="pA")[:, 0:256]
            for ct in range(NT):
                nc.tensor.matmul(
                    ps[:],
                    xvT[:, ct, ts(lt, P)],
                    wv_bf[ct][:, ts(b, 256)],
                    start=(ct == 0),
                    stop=(ct == NT - 1),
                )
            nc.vector.tensor_tensor(
                v_sb[lt][:, ts(b, 256)], ps[:], bvb[:, ts(b, 256)], ADD
            )
        return emit

    def emit_vp(state):
        hp, e0, e1, s0, s1 = state
        r0 = sums.tile([P, NT], F32, name="r")
        r1 = sums.tile([P, NT], F32, name="r")
        nc.vector.reciprocal(r0[:], s0[:])
        nc.vector.reciprocal(r1[:], s1[:])
        vp = vppool.tile([P, NT, P], BF16, name="vp")
        for kt in range(NT):
            nc.vector.tensor_scalar_mul(
                vp[:, kt, 0:64],
                v_sb[kt][:, hp * P : hp * P + 64],
                r0[:, kt : kt + 1],
            )
            nc.vector.tensor_scalar_mul(
                vp[:, kt, 64:128],
                v_sb[kt][:, hp * P + 64 : hp * P + 128],
                r1[:, kt : kt + 1],
            )
        return (hp, e0, e1, vp)

    # Fused per-kt scores(hp) + av(prev) + next head-pair's projection
    # chunks: the av/proj matmuls (independent of this hp's exps) fill the
    # PE gaps while ACT chews through the exps, instead of leaving the
    # scores stretch ACT-bound.
    def emit_scores_av(hp, qt, kt_t, prev, fill, self_av=False):
        if prev is not None:
            phD, pe0, pe1, pvp = prev
            # one bank per qc: A-half rows 0:64, B-half rows 64:128 (the
            # two groups share the bank on disjoint partition ranges)
            avps = [pairp.tile([P, 512], F32, name="pA") for _ in range(2)]
        e0 = epool.tile([P, NT, L], BF16, name="e")
        e1 = epool.tile([P, NT, L], BF16, name="e")
        s0 = sums.tile([P, NT], F32, name="esum")
        s1 = sums.tile([P, NT], F32, name="esum")
        if self_av:
            # last head pair: its own av is folded in per-kt at lag 2 (exp
            # sums are complete per-kt), so no bare trailing av pass
            savps = [pairp.tile([P, 512], F32, name="pA") for _ in range(2)]
            r0 = sums.tile([P, NT], F32, name="r")
            r1 = sums.tile([P, NT], F32, name="r")
            svp = vppool.tile([P, NT, P], BF16, name="vp")

            def self_av_kt(kt):
                for qc in range(2):
                    nc.tensor.matmul(
                        savps[qc][0:64, :],
                        svp[:, kt, 0:64],
                        e0[:, kt, ts(qc, 512)],
                        start=(kt == 0),
                        stop=(kt == NT - 1),
                        skip_group_check=True,
                    )
                    nc.tensor.matmul(
                        savps[qc][64:128, :],
                        svp[:, kt, 64:128],
                        e1[:, kt, ts(qc, 512)],
                        start=(kt == 0),
                        stop=(kt == NT - 1),
                        skip_group_check=True,
                    )

            def self_vp_kt(kt):
                nc.vector.reciprocal(r0[:, kt : kt + 1], s0[:, kt : kt + 1])
                nc.vector.reciprocal(r1[:, kt : kt + 1], s1[:, kt : kt + 1])
                nc.vector.tensor_scalar_mul(
                    svp[:, kt, 0:64],
                    v_sb[kt][:, hp * P : hp * P + 64],
                    r0[:, kt : kt + 1],
                )
                nc.vector.tensor_scalar_mul(
                    svp[:, kt, 64:128],
                    v_sb[kt][:, hp * P + 64 : hp * P + 128],
                    r1[:, kt : kt + 1],
                )
        for kt in range(NT):
            st0 = stq.tile([P, L], F32, name="st")
            st1 = stq.tile([P, L], F32, name="st")
            for qc in range(2):
                nc.tensor.matmul(
                    st0[:, ts(qc, 512)],
                    kt_t[0:64, ts(kt, P)],
                    qt[0:64, ts(qc, 512)],
                    start=True,
                    stop=True,
                )
                nc.tensor.matmul(
                    st1[:, ts(qc, 512)],
                    kt_t[64:128, ts(kt, P)],
                    qt[64:128, ts(qc, 512)],
                    start=True,
                    stop=True,
                )
            if prev is not None:
                for qc in range(2):
                    av = avps[qc]
                    nc.tensor.matmul(
                        av[0:64, :],
                        pvp[:, kt, 0:64],
                        pe0[:, kt, ts(qc, 512)],
                        start=(kt == 0),
                        stop=(kt == NT - 1),
                        skip_group_check=True,
                    )
                    nc.tensor.matmul(
                        av[64:128, :],
                        pvp[:, kt, 64:128],
                        pe1[:, kt, ts(qc, 512)],
                        start=(kt == 0),
                        stop=(kt == NT - 1),
                        skip_group_check=True,
                    )
            if fill:
                n = len(fill)
                for c in fill[kt * n // NT : (kt + 1) * n // NT]:
                    c()
            if self_av and kt >= 2:
                self_av_kt(kt - 2)
            nc.scalar.activation(
                e0[:, kt, :], st0[:], EXP, scale=SCALE,
                accum_out=s0[:, kt : kt + 1],
            )
            nc.scalar.activation(
                e1[:, kt, :], st1[:], EXP, scale=SCALE,
                accum_out=s1[:, kt : kt + 1],
            )
            if self_av:
                self_vp_kt(kt)
        if prev is not None:
            for qc in range(2):
                nc.vector.tensor_copy(ot_sb[phD][:, ts(qc, 512)], avps[qc][:])
        if self_av:
            for kt in range(NT - 2, NT):
                self_av_kt(kt)
            for qc in range(2):
                nc.vector.tensor_copy(ot_sb[hp][:, ts(qc, 512)], savps[qc][:])
        return (hp, e0, e1, s0, s1)

    def emit_av(prev):
        hp, e0, e1, vp = prev
        for qc in range(2):
            av = pairp.tile([P, 512], F32, name="pA")
            for kt in range(NT):
                nc.tensor.matmul(
                    av[0:64, :],
                    vp[:, kt, 0:64],
                    e0[:, kt, ts(qc, 512)],
                    start=(kt == 0),
                    stop=(kt == NT - 1),
                    skip_group_check=True,
                )
                nc.tensor.matmul(
                    av[64:128, :],
                    vp[:, kt, 64:128],
                    e1[:, kt, ts(qc, 512)],
                    start=(kt == 0),
                    stop=(kt == NT - 1),
                    skip_group_check=True,
                )
            nc.vector.tensor_copy(ot_sb[hp][:, ts(qc, 512)], av[:])

    def outproj():
        # full 8-term contraction + bias add straight to y
        for lt in range(NT):
            yt = qtkt.tile([P, L], BF16, name="qt")
            for nc2 in range(2):
                ps = pairp.tile([P, 512], F32, name="pA")
                for dt in range(NT):
                    nc.tensor.matmul(
                        ps[:],
                        ot_sb[dt][:, ts(lt, P)],
                        wo_bf[dt][:, ts(nc2, 512)],
                        start=(dt == 0),
                        stop=(dt == NT - 1),
                    )
                nc.vector.tensor_tensor(
                    yt[:, ts(nc2, 512)], ps[:], bob[:, ts(nc2, 512)], ADD
                )
            nc.sync.dma_start(t["y"][ts(lt, P), :], yt[:])

    # ---- main loop -------------------------------------------------------
    # proj for hp0 emitted up front; proj(hp+1)/(hp+2) chunks interleave
    # into fused(hp) so the combines land on DVE before consumption and the
    # otherwise ACT-paced hp0 gets extra PE fill.
    qt0, kt0, chunks0 = proj_chunks(0)
    for c in chunks0:
        c()
    tiles = {0: (qt0, kt0)}
    # Deadline-scheduled PE fill: ("p", hp, a, b) = proj(hp) chunks[a:b]
    # (deadline: before fused(hp)); ("v", blk, a, b) = V chains[a:b] of
    # 256-col block blk (deadline: block blk before emit_vp(2*blk)).
    # Budget: ~5.9us fill capacity per ACT-paced fused hp (12.8 at hp0).
    SCHED = {
        0: [("p", 1, 0, 8), ("p", 2, 0, 6), ("v", 0, 0, 8)],
        1: [("p", 2, 6, 8), ("p", 3, 0, 8), ("v", 1, 0, 2)],
        2: [("v", 1, 2, 8), ("p", 4, 0, 2)],
        3: [("p", 4, 2, 8), ("v", 2, 0, 4)],
        4: [("v", 2, 4, 8), ("p", 5, 0, 8)],
        5: [("p", 6, 0, 8), ("v", 3, 0, 4)],
        6: [("v", 3, 4, 8), ("p", 7, 0, 8)],
        7: [],
    }
    proj_cache = {}
    prev = None
    for hp in range(NT):
        if hp + 2 < NT:
            fetch_w(hp + 2)
        fill = []
        for kind, idx, a, b in SCHED[hp]:
            if kind == "p":
                if idx not in proj_cache:
                    qtp, ktp, ch = proj_chunks(idx)
                    tiles[idx] = (qtp, ktp)
                    proj_cache[idx] = ch
                fill += proj_cache[idx][a:b]
            else:
                fill += [mk_vchain(lt, idx) for lt in range(a, b)]
        pv = emit_vp(prev) if prev is not None else None
        qt_c, kt_c = tiles.pop(hp)
        prev = emit_scores_av(
            hp, qt_c, kt_c, pv, fill, self_av=(hp == NT - 1)
        )
    outproj()


def build_nc(looped=False, reps=None, do_split=True):
    nc = bass.Bass("TRN2", debug=False, num_devices=N_CORES, num_swdge_queues=N_QUEUES)
    XDT = FP8 if USE_FP8_QK else BF16
    t = {}
    for name in ("xq", "xk"):
        t[name] = nc.dram_tensor(name, [P, NT * L], XDT, kind="ExternalInput")
    t["xv"] = nc.dram_tensor("xv", [P, NT * L], BF16, kind="ExternalInput")
    for name in ("wv", "wo"):
        t[name] = nc.dram_tensor(name, [D, D], BF16, kind="ExternalInput")
    for name in ("wqr", "wkr"):
        t[name] = nc.dram_tensor(name, [NT * P, NT * P], XDT, kind="ExternalInput")
    t["bk"] = nc.dram_tensor("bk", [D], F32, kind="ExternalInput")
    for name in ("bvb", "bob"):
        t[name] = nc.dram_tensor(name, [P, D], BF16, kind="ExternalInput")
    t["y"] = nc.dram_tensor("y", [L, D], BF16, kind="ExternalOutput")

    with tile.TileContext(nc) as tc:
        if reps is not None:
            loop_cm = tc.For_i(0, reps, 1)
        else:
            loop_cm = nullcontext()
        with loop_cm:
            with ExitStack() as ctx:
                _emit_body(nc, tc, ctx, t)

    if do_split:
        split_excess_waits(nc)
    return nc


# ---------------------------------------------------------------------------
# Runner: mirrors bass2jax.run_bass_via_pjrt's multi-core path, but keeps a
# reusable jitted callable (no donation) so repeated kernel() calls don't
# recompile.
def make_runner(nc, n_cores=N_CORES):
    import jax
    from jax.sharding import Mesh, NamedSharding, PartitionSpec
    from jax.experimental.shard_map import shard_map
    from concourse import bass2jax
    from concourse.bass2jax import _bass_exec_p, partition_id_tensor

    bass2jax.install_neuronx_cc_hook()

    partition_name = (
        nc.partition_id_tensor.name if nc.partition_id_tensor else None
    )
    in_names, out_names, out_avals, zero_outs = [], [], [], []
    for alloc in nc.m.functions[0].allocations:
        if not isinstance(alloc, mybir.MemoryLocationSet):
            continue
        name = alloc.memorylocations[0].name
        if alloc.kind == "ExternalInput":
            if name != partition_name:
                in_names.append(name)
        elif alloc.kind == "ExternalOutput":
            shape = tuple(alloc.tensor_shape)
            dtype = mybir.dt.np(alloc.dtype)
            out_names.append(name)
            out_avals.append(jax.core.ShapedArray(shape, dtype))
            zero_outs.append(np.zeros(shape, dtype))
    n_params = len(in_names)
    all_in_names = list(in_names) + list(out_names)
    if partition_name is not None:
        all_in_names.append(partition_name)

    def _body(*args):
        operands = list(args)
        if partition_name is not None:
            operands.append(partition_id_tensor())
        outs = _bass_exec_p.bind(
            *operands,
            out_avals=tuple(out_avals),
            in_names=tuple(all_in_names),
            out_names=tuple(out_names),
            lowering_input_output_aliases=(),
            sim_require_finite=True,
            sim_require_nnan=True,
            nc=nc,
        )
        return tuple(outs)

    devices = jax.devices()[:n_cores]
    mesh = Mesh(np.asarray(devices), ("core",))
    in_specs = (PartitionSpec("core"),) * (n_params + len(out_names))
    out_specs = (PartitionSpec("core"),) * len(out_names)
    fn = jax.jit(
        shard_map(
            _body, mesh=mesh, in_specs=in_specs, out_specs=out_specs,
            check_rep=False,
        ),
        keep_unused=True,
    )
    sharding = NamedSharding(mesh, PartitionSpec("core"))
    zeros_dev = [
        jax.device_put(
            np.zeros((n_cores * z.shape[0], *z.shape[1:]), z.dtype), sharding
        )
        for z in zero_outs
    ]

    def run(in_maps):
        per_core = [[np.asarray(m[n]) for n in in_names] for m in in_maps]
        concat_in = [
            np.concatenate([per_core[c][i] for c in range(n_cores)], axis=0)
            for i in range(n_params)
        ]
        args = [jax.device_put(a, sharding) for a in concat_in] + zeros_dev
        out = fn(*args)
        jax.block_until_ready(out)
        return [
            {
                n: np.asarray(out[i]).reshape(n_cores, *out_avals[i].shape)[c]
                for i, n in enumerate(out_names)
            }
            for c in range(n_cores)
        ]

    return run, fn, in_names, out_names, out_avals, sharding


_RUNNER = None


def _xt_layout(x, dtype):
    # [l, d] -> [p, ct*L + l] with d = ct*128 + p
    xt = np.ascontiguousarray(
        np.asarray(x, np.float32).T.reshape(NT, P, L).transpose(1, 0, 2)
    ).reshape(P, NT * L)
    return xt.astype(dtype)


def _in_maps_from_inputs(inputs):
    import ml_dtypes

    bf = ml_dtypes.bfloat16
    f8 = ml_dtypes.float8_e4m3
    xdt = f8 if USE_FP8_QK else bf
    wq = np.asarray(inputs["Wq"], np.float32)
    wk = np.asarray(inputs["Wk"], np.float32)
    # [hp, p(c within ct), ct, dout] so each per-hp DMA reads contiguous
    # partition lines: w[p, ct*128+do] = W[ct*128+p, hp*128+do]
    wqr = np.ascontiguousarray(
        wq.reshape(NT, P, NT, P).transpose(2, 1, 0, 3)
    ).reshape(NT * P, NT * P).astype(xdt)
    wkr = np.ascontiguousarray(
        wk.reshape(NT, P, NT, P).transpose(2, 1, 0, 3)
    ).reshape(NT * P, NT * P).astype(xdt)
    wv = np.asarray(inputs["Wv"], np.float32).astype(bf)
    wo = np.asarray(inputs["Wo"], np.float32).astype(bf)
    bvb = np.broadcast_to(
        np.asarray(inputs["bv"], np.float32).astype(bf)[None, :], (P, D)
    ).copy()
    bob = np.broadcast_to(
        np.asarray(inputs["bo"], np.float32).astype(bf)[None, :], (P, D)
    ).copy()
    maps = []
    for b in range(N_CORES):
        m = {
            "xq": _xt_layout(inputs["x_q"][b], xdt),
            "xk": _xt_layout(inputs["x_k"][b], xdt),
            "xv": _xt_layout(inputs["x_v"][b], bf),
            "wqr": wqr,
            "wkr": wkr,
            "wv": wv,
            "wo": wo,
            "bk": np.asarray(inputs["bk"], np.float32),
            "bvb": bvb,
            "bob": bob,
        }
        maps.append(m)
    return maps


def kernel(**inputs) -> np.ndarray:
    global _RUNNER
    if _RUNNER is None:
        nc = build_nc()
        _RUNNER = make_runner(nc)[0]
    in_maps = _in_maps_from_inputs(inputs)
    _RUNNER(in_maps)  # warmup: settle device state after compile/load
    results = _RUNNER(in_maps)
    out = np.stack([results[b]["y"] for b in range(N_CORES)], axis=0)
    return out.astype(np.float32)


# revision 43
# speedup vs baseline: 1.0691x; 1.0691x over previous
"""MultiHeadAttention (CLUSTERING softmax over query axis) on 8 Trainium2 cores.

Sharding: batch B=8, one batch element per NeuronCore (pure data parallel,
no collectives).

Per-core computation (L=1024, D=1024, H=16, HD=64):
  QT = (x_q @ Wq)^T            [d, l]   (bq dropped: cancels in softmax over q)
  KT = (x_k @ Wk + bk)^T       [d, l]
  V  = x_v @ Wv + bv           [l, d]
  per head h: ST_h[k, q] = QT_h . KT_h  (contraction over hd=64)
  E = exp(ST / 32)  with fused row-sums over q (free axis)
  r = 1/sums; V'_h[k, :] = V_h[k, :] * r_h[k]   (normalizer folded into V)
  OT_h[d, q] = sum_k V'_h[k, d] * E_h[k, q]
  y = OT^T @ Wo + bo           [l, d]

v9 structure (evolved from the HW-calibrated v6):
  - x_q/x_k/x_v transposed on the HOST (no xbar DMA transposes); x_q/x_k
    and Wq/Wk shipped as fp8e4 -- the Q/K projections run in fp8
    DoubleRow perf mode (2 k-tiles per matmul; measured 2x bf16 on HW,
    not the cost model's 4x). Numerically safe: Q/K errors are crushed
    by the 1/sqrt(D) score scale + softmax.
  - V/O paths stay bf16 (fp8 there would cost ~3% rel err vs 2e-2 gate).
  - The ACT engine paces the per-head-pair fused loops (~2.45us/ktile of
    exp+accum vs 2.13us of st+av matmuls), so ALL other PE work -- the
    next head pairs' Q/K projections (2-matmul chunks) and the V
    projection (32 narrow 256-col chains) -- is deadline-scheduled as
    fill inside the fused loop slots (SCHED table).
  - av(prev) rides inside the next head pair's fused loop; the last head
    pair folds its own av in at a 2-ktile lag (per-ktile reciprocal+vp).
  - bv/bo biases via host-broadcast [128, D] tiles + DVE adds in the
    PSUM evacuation (replaces 32 rank-1 matmuls); kt bias via per-
    partition tensor_scalar_add.
  - Warmup block ramps the PE p-state and pre-loads the ACT Exp table
    during the DMA lead-in.
"""

import math
from contextlib import ExitStack, nullcontext

import numpy as np

import concourse.bass as bass
import concourse.tile as tile
from concourse import mybir
from concourse.bass import ts

F32 = mybir.dt.float32
BF16 = mybir.dt.bfloat16
FP8 = mybir.dt.float8e4
EXP = mybir.ActivationFunctionType.Exp
COPY = mybir.ActivationFunctionType.Copy
ADD = mybir.AluOpType.add
DR = mybir.MatmulPerfMode.DoubleRow

L = 1024
D = 1024
P = 128
NT = 8  # 1024 / 128
N_CORES = 8
SCALE = 1.0 / math.sqrt(D)
N_QUEUES = 4
QTKT_BUFS = 6
WQK_BUFS = 4
USE_FP8_QK = True


# ---------------------------------------------------------------------------
# Workaround: this walrus build supports very few sync-wait commands per
# instruction. Tile's kernel-tail drain / barriers can carry more. Move
# excess waits onto same-engine NOPs inserted immediately before (engines
# execute their stream in order, so this preserves semantics).
def split_excess_waits(nc):
    f = nc.m.functions[0]
    ctr = 0
    for b in f.blocks:
        insts = b.instructions
        i = 0
        while i < len(insts):
            inst = insts[i]
            si = inst.sync_info
            limit = 0 if "Drain" in type(inst).__name__ else 1
            if si is not None and si.on_wait and len(si.on_wait) > limit:
                waits = list(si.on_wait)
                keep = waits[-limit:] if limit else []
                extra = waits[: len(waits) - limit]
                pos = i
                for j in range(0, len(extra), 1):
                    nop = mybir.InstNoOp(name=f"waitsplit-{ctr}", ins=[], outs=[])
                    ctr += 1
                    nop.engine = inst.engine
                    nop.bass_nofuse = True
                    nop.sync_info = mybir.SyncInfo(
                        on_wait=[extra[j]], on_update=[]
                    )
                    insts.insert(pos, nop)
                    pos += 1
                    i += 1
                inst.sync_info = mybir.SyncInfo(
                    on_wait=keep, on_update=list(si.on_update)
                )
            i += 1


# ---------------------------------------------------------------------------
def _emit_body(nc, tc, ctx, t):
    persist = ctx.enter_context(tc.tile_pool(name="persist", bufs=1))
    pairp = ctx.enter_context(tc.tile_pool(name="pairp", bufs=4, space="PSUM"))

    XDT = FP8 if USE_FP8_QK else BF16

    # ---- constants -------------------------------------------------------
    bk_sb = persist.tile([P, NT], F32, name="bk")
    bvb = persist.tile([P, D], BF16, name="bvb")
    bob = persist.tile([P, D], BF16, name="bob")

    # ---- persistent SBUF tensors ----------------------------------------
    xqT = persist.tile([P, NT, L], XDT, name="xqT")
    xkT = persist.tile([P, NT, L], XDT, name="xkT")
    xvT = persist.tile([P, NT, L], BF16, name="xvT")
    wv_bf = [persist.tile([P, D], BF16, name=f"wv{i}") for i in range(NT)]
    wo_bf = [persist.tile([P, D], BF16, name=f"wo{i}") for i in range(NT)]
    v_sb = [persist.tile([P, D], BF16, name=f"v{i}") for i in range(NT)]
    ot_sb = [persist.tile([P, D], BF16, name=f"ot{i}") for i in range(NT)]

    # ---- DMA issue order == PE consumption order -------------------------
    # q/k path first so the first projections/scores start ASAP, then the
    # V-projection inputs (needed from hp==1), then wo (needed at the tail).
    wqk = ctx.enter_context(tc.tile_pool(name="wqk", bufs=WQK_BUFS))
    wpairs = {}

    def fetch_w(hp):
        pair = []
        for tag, wd in (("q", t["wqr"]), ("k", t["wkr"])):
            w_t = wqk.tile([P, NT, P], XDT, name=f"w{tag}")
            nc.sync.dma_start(w_t[:], wd[ts(hp, P), :])
            pair.append(w_t)
        wpairs[hp] = pair

    fetch_w(0)
    nc.sync.dma_start(bk_sb[:], t["bk"].rearrange("(a p) -> p a", p=P))
    for ct in range(NT):
        nc.sync.dma_start(xqT[:, ct, :], t["xq"][:, ts(ct, L)])
        nc.sync.dma_start(xkT[:, ct, :], t["xk"][:, ts(ct, L)])
    fetch_w(1)
    # V inputs next: the vb0 fill chains (scheduled into late hp0 slots)
    # consume (wv[ct], xvT[ct]) pairs in ct order as they land
    for ct in range(NT):
        nc.sync.dma_start(wv_bf[ct][:], t["wv"][ts(ct, P), :])
        nc.sync.dma_start(xvT[:, ct, :], t["xv"][:, ts(ct, L)])
    nc.sync.dma_start(bvb[:], t["bvb"][:, :])
    for i in range(NT):
        nc.sync.dma_start(wo_bf[i][:], t["wo"][ts(i, P), :])
    nc.sync.dma_start(bob[:], t["bob"][:, :])

    # ---- warmup during the DMA lead-in: ramp the PE p-state with dummy
    # matmuls and pre-load the ACT Exp table so the first real score work
    # runs at full clock with no table-load stall.
    warm = persist.tile([P, 512], BF16, name="warm")
    nc.gpsimd.memset(warm[:], 1.0)
    wdum = persist.tile([1, NT], F32, name="wdum")
    with tc.tile_pool(name="warmps", bufs=1, space="PSUM") as warmps:
        wps = warmps.tile([P, 512], F32, name="wp")
        for i in range(8):
            nc.tensor.matmul(
                wps[:], warm[:, 0:P], warm[:], start=True, stop=True
            )
        nc.scalar.activation(
            wdum[0:1, :], warm[0:1, 0:NT], EXP, scale=SCALE,
        )

    # ---- pools for the main loop ----------------------------------------
    qtkt = ctx.enter_context(tc.tile_pool(name="qtkt", bufs=QTKT_BUFS))
    epool = ctx.enter_context(tc.tile_pool(name="epool", bufs=4))
    sums = ctx.enter_context(tc.tile_pool(name="sums", bufs=4))
    vppool = ctx.enter_context(tc.tile_pool(name="vppool", bufs=2))
    stq = ctx.enter_context(tc.tile_pool(name="stq", bufs=2, space="PSUM"))

    # QT/KT projection for one head pair: contraction over din=1024 as
    # fp8 DoubleRow (4 matmuls of 2 k-tiles each per 512-wide chain).
    # Returns (qt_tile, kt_tile, chunks): 8 closures of 2 matmuls each,
    # interleaved into the previous head-pair's fused loop so the
    # PSUM->SBUF combines land on DVE well before the tiles are consumed.
    def proj_chunks(hp):
        qt_t = qtkt.tile([P, L], BF16, name="qt")
        kt_t = qtkt.tile([P, L], BF16, name="kt")
        chunks = []
        wp = wpairs.pop(hp)
        for out_t, w_t, tag in ((qt_t, wp[0], "qt"), (kt_t, wp[1], "kt")):
            for lc in range(2):
                cell = {}

                def mk(cell, w_t, out_t, tag, lc, i0):
                    def emit():
                        if i0 == 0:
                            cell["ps"] = pairp.tile([P, 512], F32, name="pA")
                        ps = cell["ps"]
                        for i in (i0, i0 + 1):
                            nc.tensor.matmul(
                                ps[:],
                                w_t[:, 2 * i : 2 * i + 2, :],
                                (xqT if tag == "qt" else xkT)[
                                    :, 2 * i : 2 * i + 2, ts(lc, 512)
                                ],
                                start=(i == 0),
                                stop=(i == 3),
                                perf_mode=DR,
                            )
                        if i0 == 2:
                            if tag == "kt":
                                nc.vector.tensor_scalar_add(
                                    out_t[:, ts(lc, 512)], ps[:],
                                    bk_sb[:, hp : hp + 1],
                                )
                            else:
                                nc.vector.tensor_copy(
                                    out_t[:, ts(lc, 512)], ps[:]
                                )
                    return emit

                chunks.append(mk(cell, w_t, out_t, tag, lc, 0))
                chunks.append(mk(cell, w_t, out_t, tag, lc, 2))
        return qt_t, kt_t, chunks

    # V[lt, b-quarter] = x_v @ Wv + bv: 32 chains of 8 matmuls x 256 cols,
    # used as deadline-scheduled PE fill inside the fused loops (block b
    # covers V columns for head pairs 2b..2b+1).
    def mk_vchain(lt, b):
        def emit():
            ps = pairp.tile([P, 512], F32, name="pA")[:, 0:256]
            for ct in range(NT):
                nc.tensor.matmul(
                    ps[:],
                    xvT[:, ct, ts(lt, P)],
                    wv_bf[ct][:, ts(b, 256)],
                    start=(ct == 0),
                    stop=(ct == NT - 1),
                )
            nc.vector.tensor_tensor(
                v_sb[lt][:, ts(b, 256)], ps[:], bvb[:, ts(b, 256)], ADD
            )
        return emit

    def emit_vp(state):
        hp, e0, e1, s0, s1 = state
        r0 = sums.tile([P, NT], F32, name="r")
        r1 = sums.tile([P, NT], F32, name="r")
        nc.vector.reciprocal(r0[:], s0[:])
        nc.vector.reciprocal(r1[:], s1[:])
        vp = vppool.tile([P, NT, P], BF16, name="vp")
        for kt in range(NT):
            nc.vector.tensor_scalar_mul(
                vp[:, kt, 0:64],
                v_sb[kt][:, hp * P : hp * P + 64],
                r0[:, kt : kt + 1],
            )
            nc.vector.tensor_scalar_mul(
                vp[:, kt, 64:128],
                v_sb[kt][:, hp * P + 64 : hp * P + 128],
                r1[:, kt : kt + 1],
            )
        return (hp, e0, e1, vp)

    # Fused per-kt scores(hp) + av(prev) + next head-pair's projection
    # chunks: the av/proj matmuls (independent of this hp's exps) fill the
    # PE gaps while ACT chews through the exps, instead of leaving the
    # scores stretch ACT-bound.
    def emit_scores_av(hp, qt, kt_t, prev, fill, self_av=False):
        if prev is not None:
            phD, pe0, pe1, pvp = prev
            # one bank per qc: A-half rows 0:64, B-half rows 64:128 (the
            # two groups share the bank on disjoint partition ranges)
            avps = [pairp.tile([P, 512], F32, name="pA") for _ in range(2)]
        e0 = epool.tile([P, NT, L], BF16, name="e")
        e1 = epool.tile([P, NT, L], BF16, name="e")
        s0 = sums.tile([P, NT], F32, name="esum")
        s1 = sums.tile([P, NT], F32, name="esum")
        if self_av:
            # last head pair: its own av is folded in per-kt at lag 2 (exp
            # sums are complete per-kt), so no bare trailing av pass
            savps = [pairp.tile([P, 512], F32, name="pA") for _ in range(2)]
            r0 = sums.tile([P, NT], F32, name="r")
            r1 = sums.tile([P, NT], F32, name="r")
            svp = vppool.tile([P, NT, P], BF16, name="vp")

            def self_av_kt(kt):
                for qc in range(2):
                    nc.tensor.matmul(
                        savps[qc][0:64, :],
                        svp[:, kt, 0:64],
                        e0[:, kt, ts(qc, 512)],
                        start=(kt == 0),
                        stop=(kt == NT - 1),
                        skip_group_check=True,
                    )
                    nc.tensor.matmul(
                        savps[qc][64:128, :],
                        svp[:, kt, 64:128],
                        e1[:, kt, ts(qc, 512)],
                        start=(kt == 0),
                        stop=(kt == NT - 1),
                        skip_group_check=True,
                    )

            def self_vp_kt(kt):
                nc.vector.reciprocal(r0[:, kt : kt + 1], s0[:, kt : kt + 1])
                nc.vector.reciprocal(r1[:, kt : kt + 1], s1[:, kt : kt + 1])
                nc.vector.tensor_scalar_mul(
                    svp[:, kt, 0:64],
                    v_sb[kt][:, hp * P : hp * P + 64],
                    r0[:, kt : kt + 1],
                )
                nc.vector.tensor_scalar_mul(
                    svp[:, kt, 64:128],
                    v_sb[kt][:, hp * P + 64 : hp * P + 128],
                    r1[:, kt : kt + 1],
                )
        for kt in range(NT):
            st0 = stq.tile([P, L], F32, name="st")
            st1 = stq.tile([P, L], F32, name="st")
            for qc in range(2):
                nc.tensor.matmul(
                    st0[:, ts(qc, 512)],
                    kt_t[0:64, ts(kt, P)],
                    qt[0:64, ts(qc, 512)],
                    start=True,
                    stop=True,
                )
                nc.tensor.matmul(
                    st1[:, ts(qc, 512)],
                    kt_t[64:128, ts(kt, P)],
                    qt[64:128, ts(qc, 512)],
                    start=True,
                    stop=True,
                )
            if prev is not None:
                for qc in range(2):
                    av = avps[qc]
                    nc.tensor.matmul(
                        av[0:64, :],
                        pvp[:, kt, 0:64],
                        pe0[:, kt, ts(qc, 512)],
                        start=(kt == 0),
                        stop=(kt == NT - 1),
                        skip_group_check=True,
                    )
                    nc.tensor.matmul(
                        av[64:128, :],
                        pvp[:, kt, 64:128],
                        pe1[:, kt, ts(qc, 512)],
                        start=(kt == 0),
                        stop=(kt == NT - 1),
                        skip_group_check=True,
                    )
            if fill:
                n = len(fill)
                for c in fill[kt * n // NT : (kt + 1) * n // NT]:
                    c()
            if self_av and kt >= 2:
                self_av_kt(kt - 2)
            nc.scalar.activation(
                e0[:, kt, :], st0[:], EXP, scale=SCALE,
                accum_out=s0[:, kt : kt + 1],
            )
            nc.scalar.activation(
                e1[:, kt, :], st1[:], EXP, scale=SCALE,
                accum_out=s1[:, kt : kt + 1],
            )
            if self_av:
                self_vp_kt(kt)
        if prev is not None:
            for qc in range(2):
                nc.vector.tensor_copy(ot_sb[phD][:, ts(qc, 512)], avps[qc][:])
        if self_av:
            for kt in range(NT - 2, NT):
                self_av_kt(kt)
            for qc in range(2):
                nc.vector.tensor_copy(ot_sb[hp][:, ts(qc, 512)], savps[qc][:])
        return (hp, e0, e1, s0, s1)

    def emit_av(prev):
        hp, e0, e1, vp = prev
        for qc in range(2):
            av = pairp.tile([P, 512], F32, name="pA")
            for kt in range(NT):
                nc.tensor.matmul(
                    av[0:64, :],
                    vp[:, kt, 0:64],
                    e0[:, kt, ts(qc, 512)],
                    start=(kt == 0),
                    stop=(kt == NT - 1),
                    skip_group_check=True,
                )
                nc.tensor.matmul(
                    av[64:128, :],
                    vp[:, kt, 64:128],
                    e1[:, kt, ts(qc, 512)],
                    start=(kt == 0),
                    stop=(kt == NT - 1),
                    skip_group_check=True,
                )
            nc.vector.tensor_copy(ot_sb[hp][:, ts(qc, 512)], av[:])

    def outproj():
        # full 8-term contraction + bias add straight to y
        for lt in range(NT):
            yt = qtkt.tile([P, L], BF16, name="qt")
            for nc2 in range(2):
                ps = pairp.tile([P, 512], F32, name="pA")
                for dt in range(NT):
                    nc.tensor.matmul(
                        ps[:],
                        ot_sb[dt][:, ts(lt, P)],
                        wo_bf[dt][:, ts(nc2, 512)],
                        start=(dt == 0),
                        stop=(dt == NT - 1),
                    )
                nc.vector.tensor_tensor(
                    yt[:, ts(nc2, 512)], ps[:], bob[:, ts(nc2, 512)], ADD
                )
                nc.sync.dma_start(
                    t["y"][ts(lt, P), ts(nc2, 512)], yt[:, ts(nc2, 512)]
                )

    # ---- main loop -------------------------------------------------------
    # proj for hp0 emitted up front; proj(hp+1)/(hp+2) chunks interleave
    # into fused(hp) so the combines land on DVE before consumption and the
    # otherwise ACT-paced hp0 gets extra PE fill.
    qt0, kt0, chunks0 = proj_chunks(0)
    for c in chunks0:
        c()
    tiles = {0: (qt0, kt0)}
    # Deadline-scheduled PE fill: ("p", hp, a, b) = proj(hp) chunks[a:b]
    # (deadline: before fused(hp)); ("v", blk, a, b) = V chains[a:b] of
    # 256-col block blk (deadline: block blk before emit_vp(2*blk)).
    # Budget: ~5.9us fill capacity per ACT-paced fused hp (12.8 at hp0).
    SCHED = {
        0: [("p", 1, 0, 8), ("p", 2, 0, 6), ("v", 0, 0, 8)],
        1: [("p", 2, 6, 8), ("p", 3, 0, 8), ("v", 1, 0, 2)],
        2: [("v", 1, 2, 8), ("p", 4, 0, 2)],
        3: [("p", 4, 2, 8), ("v", 2, 0, 4)],
        4: [("v", 2, 4, 8), ("p", 5, 0, 8)],
        5: [("p", 6, 0, 8), ("v", 3, 0, 4)],
        6: [("v", 3, 4, 8), ("p", 7, 0, 8)],
        7: [],
    }
    proj_cache = {}
    prev = None
    for hp in range(NT):
        if hp + 2 < NT:
            fetch_w(hp + 2)
        fill = []
        for kind, idx, a, b in SCHED[hp]:
            if kind == "p":
                if idx not in proj_cache:
                    qtp, ktp, ch = proj_chunks(idx)
                    tiles[idx] = (qtp, ktp)
                    proj_cache[idx] = ch
                fill += proj_cache[idx][a:b]
            else:
                fill += [mk_vchain(lt, idx) for lt in range(a, b)]
        pv = emit_vp(prev) if prev is not None else None
        qt_c, kt_c = tiles.pop(hp)
        prev = emit_scores_av(
            hp, qt_c, kt_c, pv, fill, self_av=(hp == NT - 1)
        )
    outproj()


def build_nc(looped=False, reps=None, do_split=True):
    nc = bass.Bass("TRN2", debug=False, num_devices=N_CORES, num_swdge_queues=N_QUEUES)
    XDT = FP8 if USE_FP8_QK else BF16
    t = {}
    for name in ("xq", "xk"):
        t[name] = nc.dram_tensor(name, [P, NT * L], XDT, kind="ExternalInput")
    t["xv"] = nc.dram_tensor("xv", [P, NT * L], BF16, kind="ExternalInput")
    for name in ("wv", "wo"):
        t[name] = nc.dram_tensor(name, [D, D], BF16, kind="ExternalInput")
    for name in ("wqr", "wkr"):
        t[name] = nc.dram_tensor(name, [NT * P, NT * P], XDT, kind="ExternalInput")
    t["bk"] = nc.dram_tensor("bk", [D], F32, kind="ExternalInput")
    for name in ("bvb", "bob"):
        t[name] = nc.dram_tensor(name, [P, D], BF16, kind="ExternalInput")
    t["y"] = nc.dram_tensor("y", [L, D], BF16, kind="ExternalOutput")

    with tile.TileContext(nc) as tc:
        if reps is not None:
            loop_cm = tc.For_i(0, reps, 1)
        else:
            loop_cm = nullcontext()
        with loop_cm:
            with ExitStack() as ctx:
                _emit_body(nc, tc, ctx, t)

    if do_split:
        split_excess_waits(nc)
    return nc


# ---------------------------------------------------------------------------
# Runner: mirrors bass2jax.run_bass_via_pjrt's multi-core path, but keeps a
# reusable jitted callable (no donation) so repeated kernel() calls don't
# recompile.
def make_runner(nc, n_cores=N_CORES):
    import jax
    from jax.sharding import Mesh, NamedSharding, PartitionSpec
    from jax.experimental.shard_map import shard_map
    from concourse import bass2jax
    from concourse.bass2jax import _bass_exec_p, partition_id_tensor

    bass2jax.install_neuronx_cc_hook()

    partition_name = (
        nc.partition_id_tensor.name if nc.partition_id_tensor else None
    )
    in_names, out_names, out_avals, zero_outs = [], [], [], []
    for alloc in nc.m.functions[0].allocations:
        if not isinstance(alloc, mybir.MemoryLocationSet):
            continue
        name = alloc.memorylocations[0].name
        if alloc.kind == "ExternalInput":
            if name != partition_name:
                in_names.append(name)
        elif alloc.kind == "ExternalOutput":
            shape = tuple(alloc.tensor_shape)
            dtype = mybir.dt.np(alloc.dtype)
            out_names.append(name)
            out_avals.append(jax.core.ShapedArray(shape, dtype))
            zero_outs.append(np.zeros(shape, dtype))
    n_params = len(in_names)
    all_in_names = list(in_names) + list(out_names)
    if partition_name is not None:
        all_in_names.append(partition_name)

    def _body(*args):
        operands = list(args)
        if partition_name is not None:
            operands.append(partition_id_tensor())
        outs = _bass_exec_p.bind(
            *operands,
            out_avals=tuple(out_avals),
            in_names=tuple(all_in_names),
            out_names=tuple(out_names),
            lowering_input_output_aliases=(),
            sim_require_finite=True,
            sim_require_nnan=True,
            nc=nc,
        )
        return tuple(outs)

    devices = jax.devices()[:n_cores]
    mesh = Mesh(np.asarray(devices), ("core",))
    in_specs = (PartitionSpec("core"),) * (n_params + len(out_names))
    out_specs = (PartitionSpec("core"),) * len(out_names)
    fn = jax.jit(
        shard_map(
            _body, mesh=mesh, in_specs=in_specs, out_specs=out_specs,
            check_rep=False,
        ),
        keep_unused=True,
    )
    sharding = NamedSharding(mesh, PartitionSpec("core"))
    zeros_dev = [
        jax.device_put(
            np.zeros((n_cores * z.shape[0], *z.shape[1:]), z.dtype), sharding
        )
        for z in zero_outs
    ]

    def run(in_maps):
        per_core = [[np.asarray(m[n]) for n in in_names] for m in in_maps]
        concat_in = [
            np.concatenate([per_core[c][i] for c in range(n_cores)], axis=0)
            for i in range(n_params)
        ]
        args = [jax.device_put(a, sharding) for a in concat_in] + zeros_dev
        out = fn(*args)
        jax.block_until_ready(out)
        return [
            {
                n: np.asarray(out[i]).reshape(n_cores, *out_avals[i].shape)[c]
                for i, n in enumerate(out_names)
            }
            for c in range(n_cores)
        ]

    return run, fn, in_names, out_names, out_avals, sharding


_RUNNER = None


def _xt_layout(x, dtype):
    # [l, d] -> [p, ct*L + l] with d = ct*128 + p
    xt = np.ascontiguousarray(
        np.asarray(x, np.float32).T.reshape(NT, P, L).transpose(1, 0, 2)
    ).reshape(P, NT * L)
    return xt.astype(dtype)


def _in_maps_from_inputs(inputs):
    import ml_dtypes

    bf = ml_dtypes.bfloat16
    f8 = ml_dtypes.float8_e4m3
    xdt = f8 if USE_FP8_QK else bf
    wq = np.asarray(inputs["Wq"], np.float32)
    wk = np.asarray(inputs["Wk"], np.float32)
    # [hp, p(c within ct), ct, dout] so each per-hp DMA reads contiguous
    # partition lines: w[p, ct*128+do] = W[ct*128+p, hp*128+do]
    wqr = np.ascontiguousarray(
        wq.reshape(NT, P, NT, P).transpose(2, 1, 0, 3)
    ).reshape(NT * P, NT * P).astype(xdt)
    wkr = np.ascontiguousarray(
        wk.reshape(NT, P, NT, P).transpose(2, 1, 0, 3)
    ).reshape(NT * P, NT * P).astype(xdt)
    wv = np.asarray(inputs["Wv"], np.float32).astype(bf)
    wo = np.asarray(inputs["Wo"], np.float32).astype(bf)
    bvb = np.broadcast_to(
        np.asarray(inputs["bv"], np.float32).astype(bf)[None, :], (P, D)
    ).copy()
    bob = np.broadcast_to(
        np.asarray(inputs["bo"], np.float32).astype(bf)[None, :], (P, D)
    ).copy()
    maps = []
    for b in range(N_CORES):
        m = {
            "xq": _xt_layout(inputs["x_q"][b], xdt),
            "xk": _xt_layout(inputs["x_k"][b], xdt),
            "xv": _xt_layout(inputs["x_v"][b], bf),
            "wqr": wqr,
            "wkr": wkr,
            "wv": wv,
            "wo": wo,
            "bk": np.asarray(inputs["bk"], np.float32),
            "bvb": bvb,
            "bob": bob,
        }
        maps.append(m)
    return maps


def kernel(**inputs) -> np.ndarray:
    global _RUNNER
    if _RUNNER is None:
        nc = build_nc()
        _RUNNER = make_runner(nc)[0]
    in_maps = _in_maps_from_inputs(inputs)
    _RUNNER(in_maps)  # warmup: settle device state after compile/load
    results = _RUNNER(in_maps)
    out = np.stack([results[b]["y"] for b in range(N_CORES)], axis=0)
    return out.astype(np.float32)


# revision 44
# speedup vs baseline: 1.0894x; 1.0190x over previous
"""MultiHeadAttention (CLUSTERING softmax over query axis) on 8 Trainium2 cores.

Sharding: batch B=8, one batch element per NeuronCore (pure data parallel,
no collectives).

Per-core computation (L=1024, D=1024, H=16, HD=64):
  QT = (x_q @ Wq)^T            [d, l]   (bq dropped: cancels in softmax over q)
  KT = (x_k @ Wk + bk)^T       [d, l]
  V  = x_v @ Wv + bv           [l, d]
  per head h: ST_h[k, q] = QT_h . KT_h  (contraction over hd=64)
  E = exp(ST / 32)  with fused row-sums over q (free axis)
  r = 1/sums; V'_h[k, :] = V_h[k, :] * r_h[k]   (normalizer folded into V)
  OT_h[d, q] = sum_k V'_h[k, d] * E_h[k, q]
  y = OT^T @ Wo + bo           [l, d]

v9 structure (evolved from the HW-calibrated v6):
  - x_q/x_k/x_v transposed on the HOST (no xbar DMA transposes); x_q/x_k
    and Wq/Wk shipped as fp8e4 -- the Q/K projections run in fp8
    DoubleRow perf mode (2 k-tiles per matmul; measured 2x bf16 on HW,
    not the cost model's 4x). Numerically safe: Q/K errors are crushed
    by the 1/sqrt(D) score scale + softmax.
  - V/O paths stay bf16 (fp8 there would cost ~3% rel err vs 2e-2 gate).
  - The ACT engine paces the per-head-pair fused loops (~2.45us/ktile of
    exp+accum vs 2.13us of st+av matmuls), so ALL other PE work -- the
    next head pairs' Q/K projections (2-matmul chunks) and the V
    projection (32 narrow 256-col chains) -- is deadline-scheduled as
    fill inside the fused loop slots (SCHED table).
  - av(prev) rides inside the next head pair's fused loop; the last head
    pair folds its own av in at a 2-ktile lag (per-ktile reciprocal+vp).
  - bv/bo biases via host-broadcast [128, D] tiles + DVE adds in the
    PSUM evacuation (replaces 32 rank-1 matmuls); kt bias via per-
    partition tensor_scalar_add.
  - Warmup block ramps the PE p-state and pre-loads the ACT Exp table
    during the DMA lead-in.
"""

import math
from contextlib import ExitStack, nullcontext

import numpy as np

import concourse.bass as bass
import concourse.tile as tile
from concourse import mybir
from concourse.bass import ts

F32 = mybir.dt.float32
BF16 = mybir.dt.bfloat16
FP8 = mybir.dt.float8e4
EXP = mybir.ActivationFunctionType.Exp
COPY = mybir.ActivationFunctionType.Copy
ADD = mybir.AluOpType.add
DR = mybir.MatmulPerfMode.DoubleRow

L = 1024
D = 1024
P = 128
NT = 8  # 1024 / 128
N_CORES = 8
SCALE = 1.0 / math.sqrt(D)
N_QUEUES = 4
QTKT_BUFS = 6
WQK_BUFS = 4
USE_FP8_QK = True


# ---------------------------------------------------------------------------
# Workaround: this walrus build supports very few sync-wait commands per
# instruction. Tile's kernel-tail drain / barriers can carry more. Move
# excess waits onto same-engine NOPs inserted immediately before (engines
# execute their stream in order, so this preserves semantics).
def split_excess_waits(nc):
    f = nc.m.functions[0]
    ctr = 0
    for b in f.blocks:
        insts = b.instructions
        i = 0
        while i < len(insts):
            inst = insts[i]
            si = inst.sync_info
            limit = 0 if "Drain" in type(inst).__name__ else 1
            if si is not None and si.on_wait and len(si.on_wait) > limit:
                waits = list(si.on_wait)
                keep = waits[-limit:] if limit else []
                extra = waits[: len(waits) - limit]
                pos = i
                for j in range(0, len(extra), 1):
                    nop = mybir.InstNoOp(name=f"waitsplit-{ctr}", ins=[], outs=[])
                    ctr += 1
                    nop.engine = inst.engine
                    nop.bass_nofuse = True
                    nop.sync_info = mybir.SyncInfo(
                        on_wait=[extra[j]], on_update=[]
                    )
                    insts.insert(pos, nop)
                    pos += 1
                    i += 1
                inst.sync_info = mybir.SyncInfo(
                    on_wait=keep, on_update=list(si.on_update)
                )
            i += 1


# ---------------------------------------------------------------------------
def _emit_body(nc, tc, ctx, t):
    persist = ctx.enter_context(tc.tile_pool(name="persist", bufs=1))
    pairp = ctx.enter_context(tc.tile_pool(name="pairp", bufs=4, space="PSUM"))

    XDT = FP8 if USE_FP8_QK else BF16

    # ---- constants -------------------------------------------------------
    bk_sb = persist.tile([P, NT], F32, name="bk")
    bvb = persist.tile([P, D], BF16, name="bvb")
    bob = persist.tile([P, D], BF16, name="bob")

    # ---- persistent SBUF tensors ----------------------------------------
    xqT = persist.tile([P, NT, L], XDT, name="xqT")
    xkT = persist.tile([P, NT, L], XDT, name="xkT")
    xvT = persist.tile([P, NT, L], BF16, name="xvT")
    wv_bf = [persist.tile([P, D], BF16, name=f"wv{i}") for i in range(NT)]
    wo_bf = [persist.tile([P, D], BF16, name=f"wo{i}") for i in range(NT)]
    v_sb = [persist.tile([P, D], BF16, name=f"v{i}") for i in range(NT)]
    ot_sb = [persist.tile([P, D], BF16, name=f"ot{i}") for i in range(NT)]

    # ---- DMA issue order == PE consumption order -------------------------
    # q/k path first so the first projections/scores start ASAP, then the
    # V-projection inputs (needed from hp==1), then wo (needed at the tail).
    wqk = ctx.enter_context(tc.tile_pool(name="wqk", bufs=WQK_BUFS))
    wpairs = {}

    def fetch_w(hp):
        pair = []
        for tag, wd in (("q", t["wqr"]), ("k", t["wkr"])):
            w_t = wqk.tile([P, NT, P], XDT, name=f"w{tag}")
            nc.sync.dma_start(w_t[:], wd[ts(hp, P), :])
            pair.append(w_t)
        wpairs[hp] = pair

    fetch_w(0)
    nc.sync.dma_start(bk_sb[:], t["bk"].rearrange("(a p) -> p a", p=P))
    for ct in range(NT):
        nc.sync.dma_start(xqT[:, ct, :], t["xq"][:, ts(ct, L)])
        nc.sync.dma_start(xkT[:, ct, :], t["xk"][:, ts(ct, L)])
    fetch_w(1)
    # V inputs next: the vb0 fill chains (scheduled into late hp0 slots)
    # consume (wv[ct], xvT[ct]) pairs in ct order as they land
    for ct in range(NT):
        nc.sync.dma_start(wv_bf[ct][:], t["wv"][ts(ct, P), :])
        nc.sync.dma_start(xvT[:, ct, :], t["xv"][:, ts(ct, L)])
    nc.sync.dma_start(bvb[:], t["bvb"][:, :])
    for i in range(NT):
        nc.sync.dma_start(wo_bf[i][:], t["wo"][ts(i, P), :])
    nc.sync.dma_start(bob[:], t["bob"][:, :])

    # ---- warmup during the DMA lead-in: ramp the PE p-state with dummy
    # matmuls and pre-load the ACT Exp table so the first real score work
    # runs at full clock with no table-load stall.
    warm = persist.tile([P, 512], BF16, name="warm")
    nc.gpsimd.memset(warm[:], 1.0)
    wdum = persist.tile([1, NT], F32, name="wdum")
    with tc.tile_pool(name="warmps", bufs=1, space="PSUM") as warmps:
        wps = warmps.tile([P, 512], F32, name="wp")
        for i in range(8):
            nc.tensor.matmul(
                wps[:], warm[:, 0:P], warm[:], start=True, stop=True
            )
        nc.scalar.activation(
            wdum[0:1, :], warm[0:1, 0:NT], EXP, scale=SCALE,
        )

    # ---- pools for the main loop ----------------------------------------
    qtkt = ctx.enter_context(tc.tile_pool(name="qtkt", bufs=QTKT_BUFS))
    epool = ctx.enter_context(tc.tile_pool(name="epool", bufs=4))
    sums = ctx.enter_context(tc.tile_pool(name="sums", bufs=4))
    vppool = ctx.enter_context(tc.tile_pool(name="vppool", bufs=2))
    stq = ctx.enter_context(tc.tile_pool(name="stq", bufs=2, space="PSUM"))

    # QT/KT projection for one head pair: contraction over din=1024 as
    # fp8 DoubleRow (4 matmuls of 2 k-tiles each per 512-wide chain).
    # Returns (qt_tile, kt_tile, chunks): 8 closures of 2 matmuls each,
    # interleaved into the previous head-pair's fused loop so the
    # PSUM->SBUF combines land on DVE well before the tiles are consumed.
    def proj_chunks(hp):
        qt_t = qtkt.tile([P, L], BF16, name="qt")
        kt_t = qtkt.tile([P, L], BF16, name="kt")
        chunks = []
        wp = wpairs.pop(hp)
        for out_t, w_t, tag in ((qt_t, wp[0], "qt"), (kt_t, wp[1], "kt")):
            for lc in range(2):
                cell = {}

                def mk(cell, w_t, out_t, tag, lc, i0):
                    def emit():
                        if i0 == 0:
                            cell["ps"] = pairp.tile([P, 512], F32, name="pA")
                        ps = cell["ps"]
                        for i in (i0, i0 + 1):
                            nc.tensor.matmul(
                                ps[:],
                                w_t[:, 2 * i : 2 * i + 2, :],
                                (xqT if tag == "qt" else xkT)[
                                    :, 2 * i : 2 * i + 2, ts(lc, 512)
                                ],
                                start=(i == 0),
                                stop=(i == 3),
                                perf_mode=DR,
                            )
                        if i0 == 2:
                            if tag == "kt":
                                nc.vector.tensor_scalar_add(
                                    out_t[:, ts(lc, 512)], ps[:],
                                    bk_sb[:, hp : hp + 1],
                                )
                            else:
                                nc.vector.tensor_copy(
                                    out_t[:, ts(lc, 512)], ps[:]
                                )
                    return emit

                chunks.append(mk(cell, w_t, out_t, tag, lc, 0))
                chunks.append(mk(cell, w_t, out_t, tag, lc, 2))
        return qt_t, kt_t, chunks

    # V[lt, b-quarter] = x_v @ Wv + bv: 32 chains of 8 matmuls x 256 cols,
    # used as deadline-scheduled PE fill inside the fused loops (block b
    # covers V columns for head pairs 2b..2b+1).
    def mk_vchain(lt, b):
        def emit():
            ps = pairp.tile([P, 512], F32, name="pA")[:, 0:256]
            for ct in range(NT):
                nc.tensor.matmul(
                    ps[:],
                    xvT[:, ct, ts(lt, P)],
                    wv_bf[ct][:, ts(b, 256)],
                    start=(ct == 0),
                    stop=(ct == NT - 1),
                )
            nc.vector.tensor_tensor(
                v_sb[lt][:, ts(b, 256)], ps[:], bvb[:, ts(b, 256)], ADD
            )
        return emit

    def emit_vp(state):
        hp, e0, e1, s0, s1 = state
        r0 = sums.tile([P, NT], F32, name="r")
        r1 = sums.tile([P, NT], F32, name="r")
        nc.vector.reciprocal(r0[:], s0[:])
        nc.vector.reciprocal(r1[:], s1[:])
        vp = vppool.tile([P, NT, P], BF16, name="vp")
        for kt in range(NT):
            nc.vector.tensor_scalar_mul(
                vp[:, kt, 0:64],
                v_sb[kt][:, hp * P : hp * P + 64],
                r0[:, kt : kt + 1],
            )
            nc.vector.tensor_scalar_mul(
                vp[:, kt, 64:128],
                v_sb[kt][:, hp * P + 64 : hp * P + 128],
                r1[:, kt : kt + 1],
            )
        return (hp, e0, e1, vp)

    # Fused per-kt scores(hp) + av(prev) + next head-pair's projection
    # chunks: the av/proj matmuls (independent of this hp's exps) fill the
    # PE gaps while ACT chews through the exps, instead of leaving the
    # scores stretch ACT-bound.
    def emit_scores_av(hp, qt, kt_t, prev, fill, self_av=False):
        if prev is not None:
            phD, pe0, pe1, pvp = prev
            # one bank per qc: A-half rows 0:64, B-half rows 64:128 (the
            # two groups share the bank on disjoint partition ranges)
            avps = [pairp.tile([P, 512], F32, name="pA") for _ in range(2)]
        e0 = epool.tile([P, NT, L], BF16, name="e")
        e1 = epool.tile([P, NT, L], BF16, name="e")
        s0 = sums.tile([P, NT], F32, name="esum")
        s1 = sums.tile([P, NT], F32, name="esum")
        if self_av:
            # last head pair: its own av is folded in per-kt at lag 2 (exp
            # sums are complete per-kt), so no bare trailing av pass
            savps = [pairp.tile([P, 512], F32, name="pA") for _ in range(2)]
            r0 = sums.tile([P, NT], F32, name="r")
            r1 = sums.tile([P, NT], F32, name="r")
            svp = vppool.tile([P, NT, P], BF16, name="vp")

            def self_av_kt(kt):
                for qc in range(2):
                    nc.tensor.matmul(
                        savps[qc][0:64, :],
                        svp[:, kt, 0:64],
                        e0[:, kt, ts(qc, 512)],
                        start=(kt == 0),
                        stop=(kt == NT - 1),
                        skip_group_check=True,
                    )
                    nc.tensor.matmul(
                        savps[qc][64:128, :],
                        svp[:, kt, 64:128],
                        e1[:, kt, ts(qc, 512)],
                        start=(kt == 0),
                        stop=(kt == NT - 1),
                        skip_group_check=True,
                    )

            def self_vp_kt(kt):
                nc.vector.reciprocal(r0[:, kt : kt + 1], s0[:, kt : kt + 1])
                nc.vector.reciprocal(r1[:, kt : kt + 1], s1[:, kt : kt + 1])
                nc.vector.tensor_scalar_mul(
                    svp[:, kt, 0:64],
                    v_sb[kt][:, hp * P : hp * P + 64],
                    r0[:, kt : kt + 1],
                )
                nc.vector.tensor_scalar_mul(
                    svp[:, kt, 64:128],
                    v_sb[kt][:, hp * P + 64 : hp * P + 128],
                    r1[:, kt : kt + 1],
                )
        for kt in range(NT):
            st0 = stq.tile([P, L], F32, name="st")
            st1 = stq.tile([P, L], F32, name="st")
            for qc in range(2):
                nc.tensor.matmul(
                    st0[:, ts(qc, 512)],
                    kt_t[0:64, ts(kt, P)],
                    qt[0:64, ts(qc, 512)],
                    start=True,
                    stop=True,
                )
                nc.tensor.matmul(
                    st1[:, ts(qc, 512)],
                    kt_t[64:128, ts(kt, P)],
                    qt[64:128, ts(qc, 512)],
                    start=True,
                    stop=True,
                )
            if prev is not None:
                for qc in range(2):
                    av = avps[qc]
                    nc.tensor.matmul(
                        av[0:64, :],
                        pvp[:, kt, 0:64],
                        pe0[:, kt, ts(qc, 512)],
                        start=(kt == 0),
                        stop=(kt == NT - 1),
                        skip_group_check=True,
                    )
                    nc.tensor.matmul(
                        av[64:128, :],
                        pvp[:, kt, 64:128],
                        pe1[:, kt, ts(qc, 512)],
                        start=(kt == 0),
                        stop=(kt == NT - 1),
                        skip_group_check=True,
                    )
            if fill:
                n = len(fill)
                for c in fill[kt * n // NT : (kt + 1) * n // NT]:
                    c()
            if self_av and kt >= 2:
                self_av_kt(kt - 2)
            nc.scalar.activation(
                e0[:, kt, :], st0[:], EXP, scale=SCALE,
                accum_out=s0[:, kt : kt + 1],
            )
            nc.scalar.activation(
                e1[:, kt, :], st1[:], EXP, scale=SCALE,
                accum_out=s1[:, kt : kt + 1],
            )
            if self_av:
                self_vp_kt(kt)
        if prev is not None:
            for qc in range(2):
                nc.vector.tensor_copy(ot_sb[phD][:, ts(qc, 512)], avps[qc][:])
        if self_av:
            for kt in range(NT - 2, NT):
                self_av_kt(kt)
            for qc in range(2):
                nc.vector.tensor_copy(ot_sb[hp][:, ts(qc, 512)], savps[qc][:])
        return (hp, e0, e1, s0, s1)

    def emit_av(prev):
        hp, e0, e1, vp = prev
        for qc in range(2):
            av = pairp.tile([P, 512], F32, name="pA")
            for kt in range(NT):
                nc.tensor.matmul(
                    av[0:64, :],
                    vp[:, kt, 0:64],
                    e0[:, kt, ts(qc, 512)],
                    start=(kt == 0),
                    stop=(kt == NT - 1),
                    skip_group_check=True,
                )
                nc.tensor.matmul(
                    av[64:128, :],
                    vp[:, kt, 64:128],
                    e1[:, kt, ts(qc, 512)],
                    start=(kt == 0),
                    stop=(kt == NT - 1),
                    skip_group_check=True,
                )
            nc.vector.tensor_copy(ot_sb[hp][:, ts(qc, 512)], av[:])

    def outproj():
        # full 8-term contraction + bias add straight to y
        for lt in range(NT):
            yt = qtkt.tile([P, L], BF16, name="qt")
            for nc2 in range(2):
                ps = pairp.tile([P, 512], F32, name="pA")
                for dt in range(NT):
                    nc.tensor.matmul(
                        ps[:],
                        ot_sb[dt][:, ts(lt, P)],
                        wo_bf[dt][:, ts(nc2, 512)],
                        start=(dt == 0),
                        stop=(dt == NT - 1),
                    )
                nc.vector.tensor_tensor(
                    yt[:, ts(nc2, 512)], ps[:], bob[:, ts(nc2, 512)], ADD
                )
                nc.sync.dma_start(
                    t["y"][ts(lt, P), ts(nc2, 512)], yt[:, ts(nc2, 512)]
                )

    # ---- main loop -------------------------------------------------------
    # proj for hp0 emitted up front; proj(hp+1)/(hp+2) chunks interleave
    # into fused(hp) so the combines land on DVE before consumption and the
    # otherwise ACT-paced hp0 gets extra PE fill.
    qt0, kt0, chunks0 = proj_chunks(0)
    for c in chunks0:
        c()
    tiles = {0: (qt0, kt0)}
    # Deadline-scheduled PE fill: ("p", hp, a, b) = proj(hp) chunks[a:b]
    # (deadline: before fused(hp)); ("v", blk, a, b) = V chains[a:b] of
    # 256-col block blk (deadline: block blk before emit_vp(2*blk)).
    # Budget: ~5.9us fill capacity per ACT-paced fused hp (12.8 at hp0).
    SCHED = {
        0: [("p", 1, 0, 8), ("p", 2, 0, 6), ("v", 0, 0, 8)],
        1: [("p", 2, 6, 8), ("p", 3, 0, 8), ("v", 1, 0, 2)],
        2: [("v", 1, 2, 8), ("p", 4, 0, 2)],
        3: [("p", 4, 2, 8), ("v", 2, 0, 4)],
        4: [("v", 2, 4, 8), ("p", 5, 0, 8)],
        5: [("p", 6, 0, 8), ("v", 3, 0, 4)],
        6: [("v", 3, 4, 8), ("p", 7, 0, 8)],
        7: [],
    }
    proj_cache = {}
    prev = None
    for hp in range(NT):
        if hp + 2 < NT:
            fetch_w(hp + 2)
        fill = []
        for kind, idx, a, b in SCHED[hp]:
            if kind == "p":
                if idx not in proj_cache:
                    qtp, ktp, ch = proj_chunks(idx)
                    tiles[idx] = (qtp, ktp)
                    proj_cache[idx] = ch
                fill += proj_cache[idx][a:b]
            else:
                fill += [mk_vchain(lt, idx) for lt in range(a, b)]
        pv = emit_vp(prev) if prev is not None else None
        qt_c, kt_c = tiles.pop(hp)
        prev = emit_scores_av(
            hp, qt_c, kt_c, pv, fill, self_av=(hp == NT - 1)
        )
    outproj()


def build_nc(looped=False, reps=None, do_split=True):
    nc = bass.Bass("TRN2", debug=False, num_devices=N_CORES, num_swdge_queues=N_QUEUES)
    XDT = FP8 if USE_FP8_QK else BF16
    t = {}
    for name in ("xq", "xk"):
        t[name] = nc.dram_tensor(name, [P, NT * L], XDT, kind="ExternalInput")
    t["xv"] = nc.dram_tensor("xv", [P, NT * L], BF16, kind="ExternalInput")
    for name in ("wv", "wo"):
        t[name] = nc.dram_tensor(name, [D, D], BF16, kind="ExternalInput")
    for name in ("wqr", "wkr"):
        t[name] = nc.dram_tensor(name, [NT * P, NT * P], XDT, kind="ExternalInput")
    t["bk"] = nc.dram_tensor("bk", [D], F32, kind="ExternalInput")
    for name in ("bvb", "bob"):
        t[name] = nc.dram_tensor(name, [P, D], BF16, kind="ExternalInput")
    t["y"] = nc.dram_tensor("y", [L, D], BF16, kind="ExternalOutput")

    with tile.TileContext(nc) as tc:
        if reps is not None:
            # For_i carries an all-engine barrier per iteration; unroll two
            # bodies per iteration so consecutive bodies overlap (the next
            # body's front DMAs run under this body's outproj tail) and the
            # barrier cost is amortized.
            u = 2 if reps % 2 == 0 else 1
            with tc.For_i(0, reps // u, 1):
                for _ in range(u):
                    with ExitStack() as ctx:
                        _emit_body(nc, tc, ctx, t)
        else:
            with ExitStack() as ctx:
                _emit_body(nc, tc, ctx, t)

    if do_split:
        split_excess_waits(nc)
    return nc


# ---------------------------------------------------------------------------
# Runner: mirrors bass2jax.run_bass_via_pjrt's multi-core path, but keeps a
# reusable jitted callable (no donation) so repeated kernel() calls don't
# recompile.
def make_runner(nc, n_cores=N_CORES):
    import jax
    from jax.sharding import Mesh, NamedSharding, PartitionSpec
    from jax.experimental.shard_map import shard_map
    from concourse import bass2jax
    from concourse.bass2jax import _bass_exec_p, partition_id_tensor

    bass2jax.install_neuronx_cc_hook()

    partition_name = (
        nc.partition_id_tensor.name if nc.partition_id_tensor else None
    )
    in_names, out_names, out_avals, zero_outs = [], [], [], []
    for alloc in nc.m.functions[0].allocations:
        if not isinstance(alloc, mybir.MemoryLocationSet):
            continue
        name = alloc.memorylocations[0].name
        if alloc.kind == "ExternalInput":
            if name != partition_name:
                in_names.append(name)
        elif alloc.kind == "ExternalOutput":
            shape = tuple(alloc.tensor_shape)
            dtype = mybir.dt.np(alloc.dtype)
            out_names.append(name)
            out_avals.append(jax.core.ShapedArray(shape, dtype))
            zero_outs.append(np.zeros(shape, dtype))
    n_params = len(in_names)
    all_in_names = list(in_names) + list(out_names)
    if partition_name is not None:
        all_in_names.append(partition_name)

    def _body(*args):
        operands = list(args)
        if partition_name is not None:
            operands.append(partition_id_tensor())
        outs = _bass_exec_p.bind(
            *operands,
            out_avals=tuple(out_avals),
            in_names=tuple(all_in_names),
            out_names=tuple(out_names),
            lowering_input_output_aliases=(),
            sim_require_finite=True,
            sim_require_nnan=True,
            nc=nc,
        )
        return tuple(outs)

    devices = jax.devices()[:n_cores]
    mesh = Mesh(np.asarray(devices), ("core",))
    in_specs = (PartitionSpec("core"),) * (n_params + len(out_names))
    out_specs = (PartitionSpec("core"),) * len(out_names)
    fn = jax.jit(
        shard_map(
            _body, mesh=mesh, in_specs=in_specs, out_specs=out_specs,
            check_rep=False,
        ),
        keep_unused=True,
    )
    sharding = NamedSharding(mesh, PartitionSpec("core"))
    zeros_dev = [
        jax.device_put(
            np.zeros((n_cores * z.shape[0], *z.shape[1:]), z.dtype), sharding
        )
        for z in zero_outs
    ]

    def run(in_maps):
        per_core = [[np.asarray(m[n]) for n in in_names] for m in in_maps]
        concat_in = [
            np.concatenate([per_core[c][i] for c in range(n_cores)], axis=0)
            for i in range(n_params)
        ]
        args = [jax.device_put(a, sharding) for a in concat_in] + zeros_dev
        out = fn(*args)
        jax.block_until_ready(out)
        return [
            {
                n: np.asarray(out[i]).reshape(n_cores, *out_avals[i].shape)[c]
                for i, n in enumerate(out_names)
            }
            for c in range(n_cores)
        ]

    return run, fn, in_names, out_names, out_avals, sharding


_RUNNER = None


def _xt_layout(x, dtype):
    # [l, d] -> [p, ct*L + l] with d = ct*128 + p
    xt = np.ascontiguousarray(
        np.asarray(x, np.float32).T.reshape(NT, P, L).transpose(1, 0, 2)
    ).reshape(P, NT * L)
    return xt.astype(dtype)


def _in_maps_from_inputs(inputs):
    import ml_dtypes

    bf = ml_dtypes.bfloat16
    f8 = ml_dtypes.float8_e4m3
    xdt = f8 if USE_FP8_QK else bf
    wq = np.asarray(inputs["Wq"], np.float32)
    wk = np.asarray(inputs["Wk"], np.float32)
    # [hp, p(c within ct), ct, dout] so each per-hp DMA reads contiguous
    # partition lines: w[p, ct*128+do] = W[ct*128+p, hp*128+do]
    wqr = np.ascontiguousarray(
        wq.reshape(NT, P, NT, P).transpose(2, 1, 0, 3)
    ).reshape(NT * P, NT * P).astype(xdt)
    wkr = np.ascontiguousarray(
        wk.reshape(NT, P, NT, P).transpose(2, 1, 0, 3)
    ).reshape(NT * P, NT * P).astype(xdt)
    wv = np.asarray(inputs["Wv"], np.float32).astype(bf)
    wo = np.asarray(inputs["Wo"], np.float32).astype(bf)
    bvb = np.broadcast_to(
        np.asarray(inputs["bv"], np.float32).astype(bf)[None, :], (P, D)
    ).copy()
    bob = np.broadcast_to(
        np.asarray(inputs["bo"], np.float32).astype(bf)[None, :], (P, D)
    ).copy()
    maps = []
    for b in range(N_CORES):
        m = {
            "xq": _xt_layout(inputs["x_q"][b], xdt),
            "xk": _xt_layout(inputs["x_k"][b], xdt),
            "xv": _xt_layout(inputs["x_v"][b], bf),
            "wqr": wqr,
            "wkr": wkr,
            "wv": wv,
            "wo": wo,
            "bk": np.asarray(inputs["bk"], np.float32),
            "bvb": bvb,
            "bob": bob,
        }
        maps.append(m)
    return maps


def kernel(**inputs) -> np.ndarray:
    global _RUNNER
    if _RUNNER is None:
        nc = build_nc()
        _RUNNER = make_runner(nc)[0]
    in_maps = _in_maps_from_inputs(inputs)
    _RUNNER(in_maps)  # warmup: settle device state after compile/load
    results = _RUNNER(in_maps)
    out = np.stack([results[b]["y"] for b in range(N_CORES)], axis=0)
    return out.astype(np.float32)
